# revision 25
# baseline (speedup 1.0000x reference)
"""CrossConv2d (concat -> 3x3 conv -> BN -> +skip -> ReLU) on 8 Trainium2 cores.

Data-parallel over the fused (b*s)=32 batch axis: 4 images per core.
Per-core Bass/Tile kernel:
  - channels (64 u + 64 v = 128) live on SBUF partitions
  - inputs are zero-padded host-side to (H+2) x (W+2), flattened, and
    pre-rounded to the fp32r grid (round-half-up to 11 mantissa bits),
    so each strip stage is one fully contiguous fp32r DMA per channel
    half straight into the matmul operand tile; the 3x3 conv is 9
    shifted matmuls (lhsT = W[tap] as [C_in, C_out], fp32r)
    accumulating into PSUM over 462-pixel chunks
  - whole padded images live in two persistent SBUF tiles (ping-pong);
    u is loaded only twice per core (all 4 images share it), v refilled
    per image; output is written width-padded (W+2) and sliced
    host-side, keeping the store DMA contiguous too
  - BN scale folded into the conv weights host-side; BN shift applied as
    the ScalarE Relu bias; skip-add is one VectorE add (in-place in PSUM)
"""

import numpy as np

import concourse.bacc as bacc
import concourse.mybir as mybir
from concourse import tile
from concourse.bass_utils import run_bass_kernel_spmd

EPS = 1e-5

B, S, C1, C2, H, W = 4, 8, 64, 64, 128, 128
CC = C1 + C2  # 128 concat channels = out channels = partition count
N_CORES = 8
IMG_PER_CORE = (B * S) // N_CORES  # 4
STRIP = 32                 # output rows per strip
NSTRIPS = H // STRIP
WP = W + 2                 # padded width
HP = H + 2                 # padded height (inputs only)
HALO = STRIP + 2           # input rows staged per strip
CHUNK = 512                # one PSUM bank; image = 31x512 + 2x383 chunks (all >=256)

F32 = mybir.dt.float32
MM_DT = mybir.dt.float32r  # full-rate fp32 matmul mode

_CACHE = {}


def _build_program():
    nc = bacc.Bacc(
        "TRN2", target_bir_lowering=False, debug=False, num_devices=N_CORES
    )
    u_d = nc.dram_tensor("u", [C1, HP * WP], MM_DT, kind="ExternalInput")
    v_d = nc.dram_tensor("v", [IMG_PER_CORE, C2, HP * WP], MM_DT, kind="ExternalInput")
    w_d = nc.dram_tensor("w", [CC, 9 * CC], MM_DT, kind="ExternalInput")
    sh_d = nc.dram_tensor("shift", [CC, 1], F32, kind="ExternalInput")
    o_d = nc.dram_tensor("o", [IMG_PER_CORE, CC, H * WP], F32, kind="ExternalOutput")

    with tile.TileContext(nc) as tc:
        with (
            tc.tile_pool(name="consts", bufs=1) as cpool,
            tc.tile_pool(name="ostrip", bufs=6) as opool,
            tc.tile_pool(name="psum", bufs=8, space="PSUM") as ppool,
        ):
            # two persistent whole-image tiles (ping-pong): all 4
            # images share the same u (same b), so the u half is loaded
            # only once per tile; only the v half is re-DMA'd per image.
            # One leading pad element aligns every PSUM chunk to an even
    	    # 512 boundary (out flat q reads input at 1 + q + off).
            xr_a = cpool.tile([CC, HP * WP + 4], MM_DT)
            xr_b = cpool.tile([CC, HP * WP + 4], MM_DT)
            # cells 0 and 16901.. feed only sliced-off pad outputs, but
    	    # must be written once: copy zeros from the padded input edge.
            # Issued first so they land before the weight halves below.
            for t_ in (xr_a, xr_b):
                nc.scalar.dma_start(t_[0:C1, 0:4], u_d[:, 0:4])
                nc.scalar.dma_start(t_[C1:CC, 0:4], v_d[0, :, 0:4])
                nc.scalar.dma_start(t_[0:C1, 1 + HP * WP :], u_d[:, 0:3])
                nc.scalar.dma_start(t_[C1:CC, 1 + HP * WP :], v_d[0, :, 0:3])
            # weights split across both HWDGE queues so the first matmul
            # group isn't gated on one 590KB transfer
            w_r = cpool.tile([CC, 9 * CC], MM_DT)
            WH = (9 * CC) // 2
            nc.scalar.dma_start(w_r[:, 0:WH], w_d[:, 0:WH])
            nc.sync.dma_start(w_r[:, WH:], w_d[:, WH:])
            sh_sb = cpool.tile([CC, 1], F32)
            nc.scalar.dma_start(sh_sb[:], sh_d[:])
            NBLK = 16
            blk = [(HP * WP * k // NBLK, HP * WP * (k + 1) // NBLK)
                   for k in range(NBLK)]

            for img in range(IMG_PER_CORE):
                xr = xr_a if img % 2 == 0 else xr_b
                for b0, b1 in blk:
                    if img < 2:
                        nc.sync.dma_start(
                            xr[0:C1, 1 + b0 : 1 + b1], u_d[:, b0:b1]
                        )
                    nc.sync.dma_start(
                        xr[C1:CC, 1 + b0 : 1 + b1], v_d[img, :, b0:b1]
                    )

                # 32 x 512 + 256: covers [0, 16640) incl. junk pad columns
                NQ = H * WP
                starts = [CHUNK * k for k in range(32)] + [32 * CHUNK]
                chunks = [(st, min(st + CHUNK, NQ)) for st in starts]
                # taps-outer over groups of 3 chunks; output DMA per group
                pss = {}
                for g0 in range(0, len(chunks), 3):
                    grp = chunks[g0 : g0 + 3]
                    gq0, gq1 = grp[0][0], grp[-1][1]
                    ogrp = opool.tile([CC, 3 * CHUNK], F32, tag="og")
                    for c, _ in grp:
                        ps_g = ppool.tile([CC, CHUNK], F32, tag="ps")
                        pss[c] = ps_g
                    for t in range(9):
                        dy, dx = t // 3 - 1, t % 3 - 1
                        off = 1 + (1 + dy) * WP + dx
                        for qc0, qc1 in grp:
                            nc.tensor.matmul(
                                pss[qc0][:, 0 : qc1 - qc0],
                                w_r[:, t * CC : (t + 1) * CC],
                                xr[:, qc0 + off : qc1 + off],
                                start=(t == 0),
                                stop=(t == 8),
                            )
                    for qc0, qc1 in grp:
                        n = qc1 - qc0
                        ps = pss[qc0]
                        # skip-add: out flat q reads input flat q + WP
                        nc.vector.tensor_add(
                            ps[:, 0:n], ps[:, 0:n],
                            xr[:, 1 + qc0 + WP : 1 + qc1 + WP],
                        )
                        nc.scalar.activation(
                            ogrp[:, qc0 - gq0 : qc1 - gq0],
                            ps[:, 0:n],
                            mybir.ActivationFunctionType.Relu,
                            bias=sh_sb[:],
                            scale=1.0,
                        )
                    nc.gpsimd.dma_start(
                        o_d[img, :, gq0:gq1],
                        ogrp[:, 0 : gq1 - gq0],
                    )
    nc.compile()
    return nc


def _get_program():
    if "nc" not in _CACHE:
        _CACHE["nc"] = _build_program()
    return _CACHE["nc"]


def _round_fp32r(a):
    """Round fp32 array to the fp32r grid: half-up at 11 mantissa bits."""
    bits = np.ascontiguousarray(a, dtype=np.float32).view(np.uint32)
    r = ((bits.astype(np.uint64) + 0x800) & ~np.uint64(0xFFF)).astype(np.uint32)
    return r.view(np.float32)


def _prep_inputs(u, v, conv_w, bn_gamma, bn_beta, bn_mean, bn_var):
    u = np.asarray(u, dtype=np.float32)
    v = np.asarray(v, dtype=np.float32)
    conv_w = np.asarray(conv_w, dtype=np.float32)
    bn_gamma = np.asarray(bn_gamma, dtype=np.float32)
    bn_beta = np.asarray(bn_beta, dtype=np.float32)
    bn_mean = np.asarray(bn_mean, dtype=np.float32)
    bn_var = np.asarray(bn_var, dtype=np.float32)

    scale = bn_gamma / np.sqrt(bn_var + EPS)
    shift = (bn_beta - bn_mean * scale).astype(np.float32).reshape(CC, 1)
    wsc = (conv_w * scale[:, None, None, None]).astype(np.float32)
    # lhsT layout per tap t = ky*3+kx: w_host[i, t*CC + o] = wsc[o, i, ky, kx]
    w_host = _round_fp32r(
        np.ascontiguousarray(wsc.transpose(1, 2, 3, 0).reshape(CC, 9 * CC))
    )

    in_maps = []
    for m in range(N_CORES):
        b = m // 2
        s0 = (m % 2) * IMG_PER_CORE
        u_pad = np.zeros((C1, HP, WP), np.float32)
        u_pad[:, 1 : 1 + H, 1 : 1 + W] = u[b, 0]
        v_pad = np.zeros((IMG_PER_CORE, C2, HP, WP), np.float32)
        v_pad[:, :, 1 : 1 + H, 1 : 1 + W] = v[b, s0 : s0 + IMG_PER_CORE]
        in_maps.append(
            {
                "u": _round_fp32r(u_pad.reshape(C1, HP * WP)),
                "v": _round_fp32r(v_pad.reshape(IMG_PER_CORE, C2, HP * WP)),
                "w": w_host,
                "shift": shift,
            }
        )
    return in_maps


def _run(inputs, trace=False):
    nc = _get_program()
    in_maps = _prep_inputs(**inputs)
    res = run_bass_kernel_spmd(
        nc, in_maps, list(range(N_CORES)), trace=trace
    )
    out = np.empty((B, 1, S, CC, H, W), np.float32)
    for m in range(N_CORES):
        b = m // 2
        s0 = (m % 2) * IMG_PER_CORE
        o_pad = res.results[m]["o"].reshape(IMG_PER_CORE, CC, H, WP)
        out[b, 0, s0 : s0 + IMG_PER_CORE] = o_pad[:, :, :, 1 : 1 + W]
    return out, res


def kernel(u, v, conv_w, bn_gamma, bn_beta, bn_mean, bn_var):
    out, _ = _run(
        dict(
            u=u,
            v=v,
            conv_w=conv_w,
            bn_gamma=bn_gamma,
            bn_beta=bn_beta,
            bn_mean=bn_mean,
            bn_var=bn_var,
        )
    )
    return out
